# revision 11
# baseline (speedup 1.0000x reference)
"""Trainium2 Bass kernel for nn_BallQLoss: PointNet++-style ball query +
grouping + L1 mask loss, sharded over 8 NeuronCores.

Per core: one (batch, row-half) shard -> 2048 query rows x 4096 candidate
columns. Pipeline per 128-row block:
  PE:   P4[n,j] = 2*dot(pc_n,pc_j) - sq_j           (K=4 matmul, f32)
  ACT:  S = sign(P4 + (r^2 - sq_n))                 (+1 in-ball, -1 out, 0 tie)
  DVE:  keyed = S * (N - j); top-16 via max8/match_replace/max8
        -> first 16 in-ball indices in ascending-j order, padded w/ first
  DMA:  wrap idx to SWDGE layout (DRAM round trip), dma_gather mask rows
  DVE:  sum |mask[n,c] - mask[idx,c]| over (slot, c) per row
Final: per-core scalar partial via ones-matmul partition reduce; host sums
partials and divides by (B*N*K).
"""
import os
import sys

import numpy as np

try:
    import concourse.bass as bass
except ImportError:
    sys.path.insert(0, '/opt/trn_rl_repo')
    import concourse.bass as bass

import concourse.mybir as mybir
import concourse.tile as tile
from concourse import bacc
from concourse.bass_utils import run_bass_kernel_spmd

f32 = mybir.dt.float32
i16 = mybir.dt.int16
i32 = mybir.dt.int32

B = 4            # batches
N = 4096         # points per batch
C = 30           # mask channels
KN = 16          # neighbors per query
R2 = np.float32(0.2) * np.float32(0.2)
NCORES = 8
ROWS = 2048      # query rows per core (half a batch)
NBLK = ROWS // 128
NF = N // 512    # 512-wide column tiles per block

_PROGRAM = None


def _build_program():
    nc = bacc.Bacc("TRN2", target_bir_lowering=False, debug=False)

    lhsT_d = nc.dram_tensor("lhsT", [4, ROWS], f32, kind="ExternalInput")
    rhs_d = nc.dram_tensor("rhs", [4, N], f32, kind="ExternalInput")
    nthr_d = nc.dram_tensor("nthr", [128, NBLK], f32, kind="ExternalInput")
    nj_d = nc.dram_tensor("nj", [N], f32, kind="ExternalInput")
    mask_d = nc.dram_tensor("maskb", [N, C], f32, kind="ExternalInput")
    own_d = nc.dram_tensor("own", [ROWS, C], f32, kind="ExternalInput")
    partial_d = nc.dram_tensor("partial", [1, 1], f32, kind="ExternalOutput")

    with tile.TileContext(nc) as tc:
        with (
            tc.tile_pool(name="const", bufs=1) as const_pool,
            tc.tile_pool(name="psum", bufs=4, space="PSUM") as psum_pool,
            tc.tile_pool(name="psumf", bufs=1, space="PSUM") as psumf_pool,
            tc.tile_pool(name="sbS", bufs=2) as s_pool,
            tc.tile_pool(name="sbK", bufs=2) as k_pool,
            tc.tile_pool(name="small", bufs=3) as small_pool,
            tc.tile_pool(name="gat", bufs=3) as gat_pool,
        ):
            lhsT = const_pool.tile([4, ROWS], f32)
            nc.sync.dma_start(lhsT[:], lhsT_d[:])
            rhs = const_pool.tile([4, N], f32)
            nc.sync.dma_start(rhs[:], rhs_d[:])
            nthr = const_pool.tile([128, NBLK], f32)
            nc.sync.dma_start(nthr[:], nthr_d[:])
            nj = const_pool.tile([128, N], f32)
            nc.sync.dma_start(nj[:], bass.AP(nj_d, 0, [[0, 128], [1, N]]))
            acc_all = const_pool.tile([128, NBLK], f32)

            for blk in range(NBLK):
                S = s_pool.tile([128, N], f32)
                keyed = k_pool.tile([128, N], f32)
                for f in range(NF):
                    fs = slice(f * 512, (f + 1) * 512)
                    p = psum_pool.tile([128, 512], f32)
                    nc.tensor.matmul(p[:], lhsT[:, blk * 128:(blk + 1) * 128],
                                     rhs[:, fs])
                    nc.scalar.activation(S[:, fs], p[:],
                                         mybir.ActivationFunctionType.Sign,
                                         bias=nthr[:, blk:blk + 1], scale=1.0)
                    nc.vector.tensor_tensor(out=keyed[:, fs], in0=S[:, fs],
                                            in1=nj[:, fs],
                                            op=mybir.AluOpType.mult)

                v16 = small_pool.tile([128, KN], f32)
                nc.vector.max(v16[:, 0:8], keyed[:])
                nc.vector.match_replace(keyed[:], v16[:, 0:8], keyed[:], -1e30)
                nc.vector.max(v16[:, 8:16], keyed[:])

                # pad slots (v<=0) get slot-0's value (first in-ball index)
                m = small_pool.tile([128, KN], f32)
                nc.vector.tensor_scalar(out=m[:], in0=v16[:], scalar1=0.0,
                                        scalar2=None, op0=mybir.AluOpType.is_gt)
                negv0 = small_pool.tile([128, 1], f32)
                nc.vector.tensor_scalar(out=negv0[:], in0=v16[:, 0:1],
                                        scalar1=-1.0, scalar2=None,
                                        op0=mybir.AluOpType.mult)
                pfix = small_pool.tile([128, KN], f32)
                nc.vector.tensor_scalar(out=pfix[:], in0=m[:], scalar1=1.0,
                                        scalar2=negv0[:, 0:1],
                                        op0=mybir.AluOpType.subtract,
                                        op1=mybir.AluOpType.mult)
                vfix = small_pool.tile([128, KN], f32)
                nc.vector.tensor_tensor(out=vfix[:], in0=v16[:], in1=m[:],
                                        op=mybir.AluOpType.mult)
                nc.vector.tensor_tensor(out=vfix[:], in0=vfix[:], in1=pfix[:],
                                        op=mybir.AluOpType.add)

                # idx = N - vfix, as int32 offsets
                idxi = small_pool.tile([128, KN], i32)
                nc.vector.tensor_scalar(out=idxi[:], in0=vfix[:],
                                        scalar1=-1.0, scalar2=float(N),
                                        op0=mybir.AluOpType.mult,
                                        op1=mybir.AluOpType.add)

                # gather: one indirect DMA per slot (one offset per partition)
                G = gat_pool.tile([128, KN, C], f32)
                for s in range(KN):
                    nc.gpsimd.indirect_dma_start(
                        G[:, s, :], None, mask_d[:],
                        bass.IndirectOffsetOnAxis(ap=idxi[:, s:s + 1], axis=0))

                own = small_pool.tile([128, C], f32)
                nc.sync.dma_start(own[:], own_d[blk * 128:(blk + 1) * 128, :])
                D = gat_pool.tile([128, KN, C], f32)
                nc.vector.tensor_tensor(
                    out=D[:], in0=G[:],
                    in1=own[:].unsqueeze(1).broadcast_to((128, KN, C)),
                    op=mybir.AluOpType.subtract)
                nc.vector.reduce_sum(acc_all[:, blk:blk + 1], D[:],
                                     mybir.AxisListType.XY,
                                     apply_absolute_value=True)

            rowtot = const_pool.tile([128, 1], f32)
            nc.vector.reduce_sum(rowtot[:], acc_all[:], mybir.AxisListType.X)
            ones = const_pool.tile([128, 1], f32)
            nc.vector.memset(ones[:], 1.0)
            ptot = psumf_pool.tile([1, 1], f32)
            nc.tensor.matmul(ptot[:], rowtot[:], ones[:])
            tot = const_pool.tile([1, 1], f32)
            nc.vector.tensor_copy(tot[:], ptot[:])
            nc.sync.dma_start(partial_d[:], tot[:])

    nc.compile()
    return nc


def _get_program():
    global _PROGRAM
    if _PROGRAM is None:
        _PROGRAM = _build_program()
    return _PROGRAM


def _make_in_maps(pc: np.ndarray, mask: np.ndarray):
    pc = np.asarray(pc, np.float32)
    mask = np.asarray(mask, np.float32)
    nj = (N - np.arange(N)).astype(np.float32)
    in_maps = []
    for core in range(NCORES):
        b, h = divmod(core, 2)
        rows = slice(h * ROWS, (h + 1) * ROWS)
        pcb = pc[b]                       # (N, 3)
        sq = np.sum(pcb * pcb, axis=1)    # (N,)
        lhsT = np.concatenate([2.0 * pcb[rows].T,
                               np.ones((1, ROWS), np.float32)], axis=0)
        rhs = np.concatenate([pcb.T, -sq[None, :]], axis=0)
        nthr = (R2 - sq[rows]).reshape(NBLK, 128).T.copy()
        in_maps.append({"lhsT": np.ascontiguousarray(lhsT),
                        "rhs": np.ascontiguousarray(rhs),
                        "nthr": np.ascontiguousarray(nthr),
                        "nj": nj,
                        "maskb": np.ascontiguousarray(mask[b]),
                        "own": np.ascontiguousarray(mask[b][rows])})
    return in_maps


def _run(pc, mask, trace=False):
    nc = _get_program()
    in_maps = _make_in_maps(pc, mask)
    res = run_bass_kernel_spmd(nc, in_maps, list(range(NCORES)), trace=trace)
    total = sum(float(r["partial"][0, 0]) for r in res.results)
    loss = np.float32(total / (B * N * KN))
    return np.asarray(loss, dtype=np.float32), res


def kernel(pc, mask):
    loss, _ = _run(pc, mask)
    return loss
